# revision 1
# baseline (speedup 1.0000x reference)
"""Multi-head attention (batch=2, seq=2048, dim=256, nhead=8, head_dim=256)
distributed across 8 trn2 NeuronCores.

Sharding: the 16 (batch, head) pairs are distributed 2-per-core (cores 0-3
handle batch 0 heads 0-7, cores 4-7 batch 1). Each core computes its two
heads' projections + attention + output-projection partial; the host sums
the 4 partials per batch and adds the output bias.

On-device per core (PSUM accumulation is always fp32):
  qT/kT [d=256, s=2048] computed bf16->fp8e4m3; QK^T runs fp8 DoubleRow
  (contraction 256 in one matmul). expT via ScalarE Exp(scale=1/16) straight
  out of PSUM (scores |s|<~1, so no max-subtraction). AV in bf16.
  Denominators are computed TRANSPOSED: bf16 add-tree over expT sk-tiles,
  then per-sq-tile partition sums via tiny matmuls (ones as the moving
  operand) -> [128,1] psum -> DVE reciprocal; 1/denom is applied as a
  per-partition scalar fused into the output-projection eviction.
  Emission is software-pipelined: chunk skew (QK of c+1 before AV of c) and
  head skew (proj of head 1 before Wo of head 0). DMA issue is round-robined
  over the sync/scalar HWDGE and gpsimd SWDGE sequencers.
"""

import sys

if "/opt/trn_rl_repo" not in sys.path:
    sys.path.insert(0, "/opt/trn_rl_repo")

import numpy as np
import ml_dtypes

P = 128
S = 2048
D = 256
CHUNK = 512
CH = S // CHUNK  # 4 sq chunks
NKT = S // P     # 16 sk tiles
NHEAD = 8
NCORES = 8

_BUILT = None


def _build():
    import concourse.bacc as bacc
    import concourse.mybir as mybir
    import concourse.tile as tile
    from contextlib import ExitStack

    BF = mybir.dt.bfloat16
    FP8 = mybir.dt.float8e4
    F32 = mybir.dt.float32
    EXP = mybir.ActivationFunctionType.Exp
    DR = mybir.MatmulPerfMode.DoubleRow

    nc = bacc.Bacc(None, target_bir_lowering=False, debug=False)
    with tile.TileContext(nc) as tc:
        with ExitStack() as ctx:
            dram = ctx.enter_context(tc.tile_pool(name="dram", bufs=1, space="DRAM"))
            xt_d = dram.tile([2, P, S], BF, kind="ExternalInput", name="xt")
            wq_d = dram.tile([2, 2, P, D], BF, kind="ExternalInput", name="wq")
            wk_d = dram.tile([2, 2, P, D], BF, kind="ExternalInput", name="wk")
            wv_d = dram.tile([2, P, 2 * D], BF, kind="ExternalInput", name="wv")
            wo_d = dram.tile([2, 2, P, D], BF, kind="ExternalInput", name="wo")
            out_d = dram.tile([S, D], F32, kind="ExternalOutput", name="out")

            const = ctx.enter_context(tc.tile_pool(name="const", bufs=1))
            ones_bf = const.tile([P, 1], BF, name="ones_bf")
            nc.vector.memset(ones_bf[:], 1.0)

            xpool = ctx.enter_context(tc.tile_pool(name="xtp", bufs=1))
            wpool = ctx.enter_context(tc.tile_pool(name="wp", bufs=1))
            xt_sb = [xpool.tile([P, S], BF, name=f"xt{et}") for et in range(2)]
            w_sb = {}
            for nm, src in (("wq", wq_d), ("wk", wk_d), ("wo", wo_d)):
                for j in range(2):
                    for et in range(2):
                        w_sb[(nm, j, et)] = wpool.tile([P, D], BF, name=f"{nm}{j}{et}")
            wv_sb = [wpool.tile([P, 2 * D], BF, name=f"wv{et}") for et in range(2)]

            # ---- input DMAs: priority order (first compute needs wk/wq j0 +
            # xt chunk 0), issue round-robined over 3 DMA-capable sequencers
            dma_engines = [nc.sync, nc.scalar, nc.gpsimd]
            loads = []
            # first projection matmul needs wk(j0) + xt chunk 0: split the
            # chunk-0 transfers in half so they land on more queues sooner
            H = CHUNK // 2
            for et in range(2):
                loads.append((xt_sb[et][:, 0:H], xt_d[et, :, 0:H]))
                loads.append((xt_sb[et][:, H:CHUNK], xt_d[et, :, H:CHUNK]))
            for et in range(2):
                loads.append((w_sb[("wk", 0, et)][:], wk_d[0, et]))
                loads.append((w_sb[("wq", 0, et)][:], wq_d[0, et]))
            for c in range(1, CH):
                for et in range(2):
                    loads.append((xt_sb[et][:, c * CHUNK:(c + 1) * CHUNK],
                                  xt_d[et, :, c * CHUNK:(c + 1) * CHUNK]))
            for et in range(2):
                loads.append((wv_sb[et][:], wv_d[et]))
            for j in range(2):
                for et in range(2):
                    loads.append((w_sb[("wo", j, et)][:], wo_d[j, et]))
            for nm, src in (("wk", wk_d), ("wq", wq_d)):
                for et in range(2):
                    loads.append((w_sb[(nm, 1, et)][:], src[1, et]))
            for i, (dst, srcap) in enumerate(loads):
                dma_engines[i % 3].dma_start(out=dst, in_=srcap)

            fpool = ctx.enter_context(tc.tile_pool(name="fp", bufs=1))
            final_sb = fpool.tile([P, NKT * D], F32, name="final")

            qkpool = ctx.enter_context(tc.tile_pool(name="qkp", bufs=2))
            vpool = ctx.enter_context(tc.tile_pool(name="vp", bufs=1))
            epool = ctx.enter_context(tc.tile_pool(name="ep", bufs=3))
            tpool = ctx.enter_context(tc.tile_pool(name="tp", bufs=2))
            rpool = ctx.enter_context(tc.tile_pool(name="rp", bufs=2))
            opool = ctx.enter_context(tc.tile_pool(name="op", bufs=2))

            psA = ctx.enter_context(tc.tile_pool(name="psA", bufs=2, space="PSUM"))
            psB = ctx.enter_context(tc.tile_pool(name="psB", bufs=3, space="PSUM"))
            psD = ctx.enter_context(tc.tile_pool(name="psD", bufs=1, space="PSUM"))

            # ---- v projection for BOTH heads at once: v2[s, h*256+d] ----
            v2_sb = vpool.tile([P, NKT * 2 * D], BF, name="v2")

            def emit_v():
                for st in range(NKT):
                    ps = psB.tile([P, CHUNK], F32, tag="psB", name="ps_v")
                    for et in range(2):
                        nc.tensor.matmul(
                            ps[:],
                            lhsT=xt_sb[et][:, st * P:(st + 1) * P],
                            rhs=wv_sb[et][:],
                            start=(et == 0), stop=(et == 1),
                        )
                    nc.scalar.copy(v2_sb[:, st * 2 * D:(st + 1) * 2 * D], ps[:])

            # ---- q/k projections: qT/kT [d=256, s=2048], stored fp8e4m3 as
            # single [128, 2*S] tiles (d-tile-major halves) for DoubleRow QK.
            # Chunk-major order so QK of chunk 0 can start early.
            def emit_proj_qk(j):
                qt_sb = qkpool.tile([P, 2 * S], FP8, tag="qt", name=f"qt_{j}")
                kt_sb = qkpool.tile([P, 2 * S], FP8, tag="kt", name=f"kt_{j}")
                for c in range(CH):
                    for dst, wname in ((kt_sb, "wk"), (qt_sb, "wq")):
                        for dt in range(2):
                            ps = psB.tile([P, CHUNK], F32, tag="psB", name="ps_proj")
                            for et in range(2):
                                nc.tensor.matmul(
                                    ps[:],
                                    lhsT=w_sb[(wname, j, et)][:, dt * P:(dt + 1) * P],
                                    rhs=xt_sb[et][:, c * CHUNK:(c + 1) * CHUNK],
                                    start=(et == 0), stop=(et == 1),
                                )
                            nc.vector.tensor_copy(
                                dst[:, dt * S + c * CHUNK: dt * S + (c + 1) * CHUNK], ps[:])
                qt3 = qt_sb.rearrange("p (ko s) -> p ko s", ko=2)
                kt3 = kt_sb.rearrange("p (ko s) -> p ko s", ko=2)
                return qt3, kt3

            def emit_attn(j, qt3, kt3):
                outu_sb = [opool.tile([P, S], BF, tag=f"ou{dt}", name=f"ou{dt}_{j}")
                           for dt in range(2)]
                recipT = rpool.tile([P, NKT], F32, tag="recipT", name=f"recipT_{j}")

                def wo_cb(c):
                    emit_wo_group(j, outu_sb, recipT, c)

                def emit_qk(c):
                    E = epool.tile([P, NKT * CHUNK], BF, tag="E", name=f"E_{j}_{c}")
                    for g in range(NKT // 2):
                        ps = psA.tile([P, 2 * CHUNK], F32, tag="psA", name="ps_qk")
                        for half in range(2):
                            kt_idx = 2 * g + half
                            nc.tensor.matmul(
                                ps[:, half * CHUNK:(half + 1) * CHUNK],
                                lhsT=kt3[:, :, kt_idx * P:(kt_idx + 1) * P],
                                rhs=qt3[:, :, c * CHUNK:(c + 1) * CHUNK],
                                start=True, stop=True, perf_mode=DR,
                            )
                        nc.scalar.activation(
                            E[:, g * 2 * CHUNK:(g + 1) * 2 * CHUNK], ps[:],
                            EXP, scale=1.0 / 16.0,
                        )
                    return E

                def emit_av(c, E):
                    for dt in range(2):
                        ps = psB.tile([P, CHUNK], F32, tag="psB", name="ps_av")
                        for kt_idx in range(NKT):
                            off = kt_idx * 2 * D + j * D + dt * P
                            nc.tensor.matmul(
                                ps[:],
                                lhsT=v2_sb[:, off:off + P],
                                rhs=E[:, kt_idx * CHUNK:(kt_idx + 1) * CHUNK],
                                start=(kt_idx == 0), stop=(kt_idx == NKT - 1),
                            )
                        nc.vector.tensor_copy(
                            outu_sb[dt][:, c * CHUNK:(c + 1) * CHUNK], ps[:])

                def emit_tree(c, E):
                    t1 = tpool.tile([P, 8 * CHUNK], BF, tag="t1", name="t1")
                    nc.vector.tensor_add(t1[:], E[:, :8 * CHUNK], E[:, 8 * CHUNK:])
                    t2 = tpool.tile([P, 4 * CHUNK], BF, tag="t2", name="t2")
                    nc.vector.tensor_add(t2[:], t1[:, :4 * CHUNK], t1[:, 4 * CHUNK:])
                    t3 = tpool.tile([P, 2 * CHUNK], BF, tag="t3", name="t3")
                    nc.vector.tensor_add(t3[:], t2[:, :2 * CHUNK], t2[:, 2 * CHUNK:])
                    t4 = tpool.tile([P, CHUNK], BF, tag="t4", name="t4")
                    nc.vector.tensor_add(t4[:], t3[:, :CHUNK], t3[:, CHUNK:])
                    return t4

                def emit_densum(c, t4):
                    psd = psD.tile([P, CHUNK // P], F32, tag="psD", name="ps_d")
                    for qtr in range(CHUNK // P):
                        nc.tensor.matmul(psd[:, qtr:qtr + 1],
                                         lhsT=t4[:, qtr * P:(qtr + 1) * P],
                                         rhs=ones_bf[:], start=True, stop=True)
                    nc.vector.reciprocal(
                        recipT[:, c * (CHUNK // P):(c + 1) * (CHUNK // P)], psd[:])

                # skewed pipeline: qk(c+1) before av(c); the denominator tree
                # (DVE) is emitted before av(c) (PE) so they overlap and the
                # last chunk's 1/denom is ready before its wo group; wo for
                # chunk c trails by 2 chunks
                prev_E = emit_qk(0)
                for c in range(1, CH):
                    E_c = emit_qk(c)
                    t4 = emit_tree(c - 1, prev_E)
                    emit_av(c - 1, prev_E)
                    emit_densum(c - 1, t4)
                    if c >= 2:
                        wo_cb(c - 2)
                    prev_E = E_c
                t4 = emit_tree(CH - 1, prev_E)
                emit_av(CH - 1, prev_E)
                emit_densum(CH - 1, t4)
                wo_cb(CH - 2)
                wo_cb(CH - 1)
                return outu_sb, recipT

            def emit_wo_group(j, outu_sb, recipT, c):
                for st in range(4 * c, 4 * c + 4):
                    ps = psB.tile([P, CHUNK], F32, tag="psB", name="ps_o")
                    for dt in range(2):
                        nc.tensor.matmul(
                            ps[:, :D],
                            lhsT=outu_sb[dt][:, st * P:(st + 1) * P],
                            rhs=w_sb[("wo", j, dt)][:],
                            start=(dt == 0), stop=(dt == 1),
                        )
                    if j == 0:
                        nc.vector.tensor_scalar_mul(
                            final_sb[:, st * D:(st + 1) * D], ps[:, :D],
                            recipT[:, st:st + 1],
                        )
                    else:
                        nc.vector.scalar_tensor_tensor(
                            final_sb[:, st * D:(st + 1) * D],
                            ps[:, :D], recipT[:, st:st + 1],
                            final_sb[:, st * D:(st + 1) * D],
                            op0=mybir.AluOpType.mult, op1=mybir.AluOpType.add,
                        )
                        dma_engines[st % 3].dma_start(
                            out=out_d[st * P:(st + 1) * P, :],
                            in_=final_sb[:, st * D:(st + 1) * D],
                        )

            # head-level software pipeline (wo groups are inlined per chunk)
            qt0, kt0 = emit_proj_qk(0)
            emit_v()
            emit_attn(0, qt0, kt0)
            qt1, kt1 = emit_proj_qk(1)
            emit_attn(1, qt1, kt1)
    nc.compile()
    names = dict(xt=xt_d.name, wq=wq_d.name, wk=wk_d.name, wv=wv_d.name,
                 wo=wo_d.name, out=out_d.name)
    return nc, names


def _get_built():
    global _BUILT
    if _BUILT is None:
        _BUILT = _build()
    return _BUILT


def _prep_core_inputs(i, x, Wq, Wk, Wv, Wo, names):
    bf16 = ml_dtypes.bfloat16
    b = i // 4
    heads = [(2 * i) % NHEAD, (2 * i) % NHEAD + 1]
    xt = np.ascontiguousarray(x[b].T).reshape(2, P, S).astype(bf16)

    def head_T(W, h):  # W[h*D:(h+1)*D, :].T -> [e=256, d=256] -> [2,128,256]
        return np.ascontiguousarray(W[h * D:(h + 1) * D, :].T).reshape(2, P, D)

    wq = np.stack([head_T(Wq, h) for h in heads]).astype(bf16)
    wk = np.stack([head_T(Wk, h) for h in heads]).astype(bf16)
    # wv: both heads side by side -> [et=2, 128, 2*D]
    wv = np.concatenate([head_T(Wv, h) for h in heads], axis=2).astype(bf16)
    wo = np.stack(
        [np.ascontiguousarray(Wo[:, h * D:(h + 1) * D].T).reshape(2, P, D) for h in heads]
    ).astype(bf16)
    return {names["xt"]: xt, names["wq"]: wq, names["wk"]: wk,
            names["wv"]: wv, names["wo"]: wo}


def kernel(x, Wq, Wk, Wv, Wo, bo):
    from concourse.bass_utils import run_bass_kernel_spmd

    x = np.asarray(x, dtype=np.float32)
    Wq = np.asarray(Wq, dtype=np.float32)
    Wk = np.asarray(Wk, dtype=np.float32)
    Wv = np.asarray(Wv, dtype=np.float32)
    Wo = np.asarray(Wo, dtype=np.float32)
    bo = np.asarray(bo, dtype=np.float32)

    nc, names = _get_built()
    in_maps = [_prep_core_inputs(i, x, Wq, Wk, Wv, Wo, names) for i in range(NCORES)]
    res = run_bass_kernel_spmd(nc, in_maps, core_ids=list(range(NCORES)))

    out = np.zeros((2, S, D), dtype=np.float32)
    for b in range(2):
        acc = np.zeros((S, D), dtype=np.float32)
        for i in range(4 * b, 4 * b + 4):
            acc += res.results[i][names["out"]]
        out[b] = acc + bo[None, :]
    return out



# revision 3
# speedup vs baseline: 1.1259x; 1.1259x over previous
"""Multi-head attention (batch=2, seq=2048, dim=256, nhead=8, head_dim=256)
distributed across 8 trn2 NeuronCores.

Sharding: the 16 (batch, head) pairs are distributed 2-per-core (cores 0-3
handle batch 0 heads 0-7, cores 4-7 batch 1). Each core computes its two
heads end-to-end; the host sums the 4 partials per batch and adds the bias.

Key structure (v2):
  - Wo is folded into the v projection on the host: W' = Wo_h @ Wv_h, so the
    kernel computes v' = x @ W'^T and the AV matmul directly yields the final
    per-head output partial. No separate output-projection matmuls.
  - AV is emitted "flipped": lhsT = E tile [sk,sq], rhs = v' [sk, o], so the
    output lands as [sq, o] — matching DRAM layout and making the softmax
    denominator a per-partition scalar at eviction time.
  - The denominator rides the AV matmul as a ones-column appended to each v'
    sk-block (rhs width 257); no DVE add-tree and no tiny densum matmuls.
  - q/k projections run fp8e4m3 DoubleRow (x and Wq/Wk shipped as fp8, with
    Wq/Wk pre-scaled by 64 on the host; the exp activation scale absorbs the
    4096x score scale). QK^T also fp8 DoubleRow as before.
  - PE-queue emission interleaves QK groups of chunk c+1 (and the j1
    projections) between the AV matmuls of chunk c so the scalar engine's
    Exp never stalls the PE.
"""

import sys

if "/opt/trn_rl_repo" not in sys.path:
    sys.path.insert(0, "/opt/trn_rl_repo")

import numpy as np
import ml_dtypes

P = 128
S = 2048
D = 256
CHUNK = 512
CH = S // CHUNK  # 4 sq chunks
NKT = S // P     # 16 sk tiles
NHEAD = 8
NCORES = 8
WSCALE = 64.0
EXPSCALE = 1.0 / (16.0 * WSCALE * WSCALE)
VB = D + 1       # v' block width incl. ones column

_BUILT = None


def _build():
    import concourse.bacc as bacc
    import concourse.mybir as mybir
    import concourse.tile as tile
    from contextlib import ExitStack

    BF = mybir.dt.bfloat16
    FP8 = mybir.dt.float8e4
    F32 = mybir.dt.float32
    EXP = mybir.ActivationFunctionType.Exp
    DR = mybir.MatmulPerfMode.DoubleRow

    nc = bacc.Bacc(None, target_bir_lowering=False, debug=False)
    with tile.TileContext(nc) as tc:
        with ExitStack() as ctx:
            dram = ctx.enter_context(tc.tile_pool(name="dram", bufs=1, space="DRAM"))
            xtb_d = dram.tile([2, P, S], BF, kind="ExternalInput", name="xtb")
            xt8_d = dram.tile([P, 2 * S], FP8, kind="ExternalInput", name="xt8")
            wq8_d = dram.tile([2, P, 2 * D], FP8, kind="ExternalInput", name="wq8")
            wk8_d = dram.tile([2, P, 2 * D], FP8, kind="ExternalInput", name="wk8")
            wp_d = dram.tile([2, P, 2 * D], BF, kind="ExternalInput", name="wp")
            out_d = dram.tile([S, D], F32, kind="ExternalOutput", name="out")

            xpool = ctx.enter_context(tc.tile_pool(name="xtp", bufs=1))
            wpool = ctx.enter_context(tc.tile_pool(name="wp", bufs=1))
            xtb_sb = [xpool.tile([P, S], BF, name=f"xtb{et}") for et in range(2)]
            xt8_sb = xpool.tile([P, 2 * S], FP8, name="xt8")
            wq8_sb = [wpool.tile([P, 2 * D], FP8, name=f"wq8{j}") for j in range(2)]
            wk8_sb = [wpool.tile([P, 2 * D], FP8, name=f"wk8{j}") for j in range(2)]
            wp_sb = [wpool.tile([P, 2 * D], BF, name=f"wp{et}") for et in range(2)]

            xt8v = xt8_sb.rearrange("p (ko s) -> p ko s", ko=2)
            wq3 = [w.rearrange("p (ko d) -> p ko d", ko=2) for w in wq8_sb]
            wk3 = [w.rearrange("p (ko d) -> p ko d", ko=2) for w in wk8_sb]

            # ---- input DMAs: first compute needs wk8 j0 + xt8 chunk 0 ----
            dma_engines = [nc.sync, nc.scalar, nc.gpsimd]
            loads = []
            loads.append((wk8_sb[0][:], wk8_d[0]))
            for et in range(2):
                loads.append((xt8_sb[:, et * S:et * S + CHUNK],
                              xt8_d[:, et * S:et * S + CHUNK]))
            loads.append((wq8_sb[0][:], wq8_d[0]))
            for c in range(1, CH):
                for et in range(2):
                    o = et * S + c * CHUNK
                    loads.append((xt8_sb[:, o:o + CHUNK], xt8_d[:, o:o + CHUNK]))
            for et in range(2):
                loads.append((wp_sb[et][:], wp_d[et]))
            for c in range(CH):
                for et in range(2):
                    loads.append((xtb_sb[et][:, c * CHUNK:(c + 1) * CHUNK],
                                  xtb_d[et, :, c * CHUNK:(c + 1) * CHUNK]))
            loads.append((wk8_sb[1][:], wk8_d[1]))
            loads.append((wq8_sb[1][:], wq8_d[1]))
            for i, (dst, srcap) in enumerate(loads):
                dma_engines[i % 3].dma_start(out=dst, in_=srcap)

            fpool = ctx.enter_context(tc.tile_pool(name="fp", bufs=1))
            final_sb = fpool.tile([P, NKT * D], F32, name="final")

            vpool = ctx.enter_context(tc.tile_pool(name="vp", bufs=1))
            v2_sb = vpool.tile([P, 2 * NKT * VB], BF, name="v2")

            qkpool = ctx.enter_context(tc.tile_pool(name="qkp", bufs=2))
            epool = ctx.enter_context(tc.tile_pool(name="ep", bufs=3))
            rpool = ctx.enter_context(tc.tile_pool(name="rp", bufs=2))

            psA = ctx.enter_context(tc.tile_pool(name="psA", bufs=2, space="PSUM"))
            psB = ctx.enter_context(tc.tile_pool(name="psB", bufs=2, space="PSUM"))
            psC = ctx.enter_context(tc.tile_pool(name="psC", bufs=2, space="PSUM"))

            # ones columns (denominator) in every v' block
            for b in range(2 * NKT):
                nc.vector.memset(v2_sb[:, b * VB + D:b * VB + VB], 1.0)

            # ---- q/k projection, fp8 DoubleRow; casts split scalar/vector ----
            def emit_proj_chunk(j, c, qt_sb, kt_sb):
                for dst, w3, ceng in ((kt_sb, wk3[j], nc.scalar),
                                      (qt_sb, wq3[j], nc.vector)):
                    for dt in range(2):
                        ps = psB.tile([P, CHUNK], F32, tag="psB", name="ps_proj")
                        nc.tensor.matmul(
                            ps[:],
                            lhsT=w3[:, :, dt * P:(dt + 1) * P],
                            rhs=xt8v[:, :, c * CHUNK:(c + 1) * CHUNK],
                            start=True, stop=True, perf_mode=DR,
                        )
                        if ceng is nc.scalar:
                            nc.scalar.copy(
                                dst[:, dt * S + c * CHUNK: dt * S + (c + 1) * CHUNK],
                                ps[:])
                        else:
                            nc.vector.tensor_copy(
                                dst[:, dt * S + c * CHUNK: dt * S + (c + 1) * CHUNK],
                                ps[:])

            # ---- v' projection (Wo folded): one st tile = [128, 512] ----
            def emit_vprime_st(st):
                ps = psB.tile([P, CHUNK], F32, tag="psB", name="ps_v")
                for et in range(2):
                    nc.tensor.matmul(
                        ps[:],
                        lhsT=xtb_sb[et][:, st * P:(st + 1) * P],
                        rhs=wp_sb[et][:],
                        start=(et == 0), stop=(et == 1),
                    )
                for j in range(2):
                    blk = (2 * st + j) * VB
                    nc.vector.tensor_copy(v2_sb[:, blk:blk + D],
                                          ps[:, j * D:(j + 1) * D])

            # ---- QK group: 2 DR matmuls (sk tiles 2g, 2g+1) + Exp ----
            def emit_qk_group(qt3, kt3, c, g, E):
                ps = psA.tile([P, 2 * CHUNK], F32, tag="psA", name="ps_qk")
                for half in range(2):
                    kt_idx = 2 * g + half
                    nc.tensor.matmul(
                        ps[:, half * CHUNK:(half + 1) * CHUNK],
                        lhsT=kt3[:, :, kt_idx * P:(kt_idx + 1) * P],
                        rhs=qt3[:, :, c * CHUNK:(c + 1) * CHUNK],
                        start=True, stop=True, perf_mode=DR,
                    )
                nc.scalar.activation(
                    E[:, g * 2 * CHUNK:(g + 1) * 2 * CHUNK], ps[:],
                    EXP, scale=EXPSCALE,
                )

            # ---- AV chunk as a generator: yields after each matmul ----
            def gen_av(j, c, E):
                for qd in range(4):
                    st = c * 4 + qd
                    ps = psC.tile([P, VB], F32, tag="psC", name="ps_av")
                    for kt_idx in range(NKT):
                        nc.tensor.matmul(
                            ps[:],
                            lhsT=E[:, kt_idx * CHUNK + qd * P: kt_idx * CHUNK + (qd + 1) * P],
                            rhs=v2_sb[:, (2 * kt_idx + j) * VB: (2 * kt_idx + j + 1) * VB],
                            start=(kt_idx == 0), stop=(kt_idx == NKT - 1),
                        )
                        yield
                    rc = rpool.tile([P, 1], F32, tag="rc", name="recip")
                    nc.vector.reciprocal(rc[:], ps[:, D:D + 1])
                    fs = final_sb[:, st * D:(st + 1) * D]
                    if j == 0:
                        nc.vector.tensor_scalar_mul(fs, ps[:, :D], rc[:])
                    else:
                        nc.vector.scalar_tensor_tensor(
                            fs, ps[:, :D], rc[:], fs,
                            op0=mybir.AluOpType.mult, op1=mybir.AluOpType.add,
                        )
                        for half in range(2):
                            eng = dma_engines[(2 * st + half) % 3]
                            eng.dma_start(
                                out=out_d[st * P:(st + 1) * P, half * P:(half + 1) * P],
                                in_=final_sb[:, st * D + half * P: st * D + (half + 1) * P],
                            )

            def drive(gen, riders):
                """Interleave rider callables evenly between the generator's
                64 matmul steps."""
                n_av = 4 * NKT
                nr = len(riders)
                fired = 0
                for i, _ in enumerate(gen):
                    want = ((i + 1) * nr) // n_av
                    while fired < want:
                        riders[fired]()
                        fired += 1
                while fired < nr:
                    riders[fired]()
                    fired += 1

            # ================= emission =================
            qt_sb = [qkpool.tile([P, 2 * S], FP8, tag="qt", name=f"qt{j}")
                     for j in range(2)]
            kt_sb = [qkpool.tile([P, 2 * S], FP8, tag="kt", name=f"kt{j}")
                     for j in range(2)]
            qt3 = [t.rearrange("p (ko s) -> p ko s", ko=2) for t in qt_sb]
            kt3 = [t.rearrange("p (ko s) -> p ko s", ko=2) for t in kt_sb]

            E_tiles = {}

            def make_E(j, c):
                E_tiles[(j, c)] = epool.tile([P, NKT * CHUNK], BF, tag="E",
                                             name=f"E_{j}_{c}")
                return E_tiles[(j, c)]

            # P0: q/k proj head 0
            for c in range(CH):
                emit_proj_chunk(0, c, qt_sb[0], kt_sb[0])

            # P1: QK(j0, c0) groups with v' riders (2 st per group)
            E00 = make_E(0, 0)
            for g in range(NKT // 2):
                emit_qk_group(qt3[0], kt3[0], 0, g, E00)
                emit_vprime_st(2 * g)
                emit_vprime_st(2 * g + 1)

            # j0 steady chunks: AV(c-1) with QK(c) riders
            for c in range(1, CH):
                E_new = make_E(0, c)
                riders = [
                    (lambda g=g, c=c, E=E_new: emit_qk_group(qt3[0], kt3[0], c, g, E))
                    for g in range(NKT // 2)
                ]
                drive(gen_av(0, c - 1, E_tiles[(0, c - 1)]), riders)

            # AV(j0, 3) with riders: proj j1 chunks + QK(j1, 0) groups
            E10 = make_E(1, 0)
            riders = [
                (lambda c=c: emit_proj_chunk(1, c, qt_sb[1], kt_sb[1]))
                for c in range(CH)
            ] + [
                (lambda g=g: emit_qk_group(qt3[1], kt3[1], 0, g, E10))
                for g in range(NKT // 2)
            ]
            drive(gen_av(0, CH - 1, E_tiles[(0, CH - 1)]), riders)

            # j1 steady chunks
            for c in range(1, CH):
                E_new = make_E(1, c)
                riders = [
                    (lambda g=g, c=c, E=E_new: emit_qk_group(qt3[1], kt3[1], c, g, E))
                    for g in range(NKT // 2)
                ]
                drive(gen_av(1, c - 1, E_tiles[(1, c - 1)]), riders)

            # final AV chunk, no riders
            drive(gen_av(1, CH - 1, E_tiles[(1, CH - 1)]), [])

    nc.compile()
    names = dict(xtb=xtb_d.name, xt8=xt8_d.name, wq8=wq8_d.name,
                 wk8=wk8_d.name, wp=wp_d.name, out=out_d.name)
    return nc, names


def _get_built():
    global _BUILT
    if _BUILT is None:
        _BUILT = _build()
    return _BUILT


def _prep_core_inputs(i, x, Wq, Wk, Wv, Wo, names):
    bf16 = ml_dtypes.bfloat16
    fp8 = ml_dtypes.float8_e4m3fn
    b = i // 4
    heads = [(2 * i) % NHEAD, (2 * i) % NHEAD + 1]

    xt = np.ascontiguousarray(x[b].T)                      # [256, 2048]
    xtb = xt.reshape(2, P, S).astype(bf16)
    xt8 = np.ascontiguousarray(
        xt.reshape(2, P, S).transpose(1, 0, 2).reshape(P, 2 * S)).astype(fp8)

    def w8_head(W, h):  # lhsT fp8 DR layout [128, (ko, d)]
        wT = W[h * D:(h + 1) * D, :].T * WSCALE            # [e, d]
        return np.ascontiguousarray(
            wT.reshape(2, P, D).transpose(1, 0, 2).reshape(P, 2 * D)).astype(fp8)

    wq8 = np.stack([w8_head(Wq, h) for h in heads])
    wk8 = np.stack([w8_head(Wk, h) for h in heads])

    def wp_head(h):  # W' = Wo_h @ Wv_h; rhs layout [et, 128, o]
        Wp = Wo[:, h * D:(h + 1) * D] @ Wv[h * D:(h + 1) * D]   # [o, e]
        return Wp.T.reshape(2, P, D)                            # [et, 128, o]

    wps = [wp_head(h) for h in heads]
    wp = np.concatenate(wps, axis=2).astype(bf16)               # [2, 128, 512]
    return {names["xtb"]: xtb, names["xt8"]: xt8, names["wq8"]: wq8,
            names["wk8"]: wk8, names["wp"]: wp}


def kernel(x, Wq, Wk, Wv, Wo, bo):
    from concourse.bass_utils import run_bass_kernel_spmd

    x = np.asarray(x, dtype=np.float32)
    Wq = np.asarray(Wq, dtype=np.float32)
    Wk = np.asarray(Wk, dtype=np.float32)
    Wv = np.asarray(Wv, dtype=np.float32)
    Wo = np.asarray(Wo, dtype=np.float32)
    bo = np.asarray(bo, dtype=np.float32)

    nc, names = _get_built()
    in_maps = [_prep_core_inputs(i, x, Wq, Wk, Wv, Wo, names) for i in range(NCORES)]
    res = run_bass_kernel_spmd(nc, in_maps, core_ids=list(range(NCORES)))

    out = np.zeros((2, S, D), dtype=np.float32)
    for b in range(2):
        acc = np.zeros((S, D), dtype=np.float32)
        for i in range(4 * b, 4 * b + 4):
            acc += res.results[i][names["out"]]
        out[b] = acc + bo[None, :]
    return out
